# revision 14
# baseline (speedup 1.0000x reference)
"""MiniMHSA Trainium2 kernel: 8 NeuronCores, shard = (batch n, head-group).

Reference computes, per batch n:
  qkv = x @ W_qkv.T + b_qkv ; split into q,k,v heads (H=16, HD=64)
  scores = (q @ k.T) / sqrt(HD), masked keys -> -1e9, softmax, @ v
  out = attn_out @ W_out.T + b_out

Core c handles n = c//2 and head-group hg = c%2 (8 heads each). Device-side
dataflow (all matmuls float32r = TF32-like, 4x faster than fp32 on the PE):

  xT [D, L] (host-transposed), W slices host-transposed/scaled:
  1) qkT  = Wqk.T @ x.T   -> q,k transposed [64*16cols, L] (+ bias per-partition)
  2) v    = x @ Wv        -> v natural [L, 512] + ones column (softmax denom)
  3) S^T  = kT.T @ qT     -> scores with k on partitions; mask = per-partition
     bias of the exp ACTIVATE (masked rows underflow to 0); no max-subtraction
     (scores are O(5), exp is safe in fp32)
  4) O^T  = v'.T @ P^T    -> [65, L] accumulated over k chunks; row 64 = sums
  5) normalize via reciprocal + rank-1 ones-outer-product broadcast
  6) y    = otn.T @ Wo + b_out (partial over this core's heads)

Host sums the two head-group partials per batch.
"""
import sys

sys.path.insert(0, '/opt/trn_rl_repo')


import numpy as np

_KERNEL_CACHE = {}


def _split_excess_waits(nc):
    """Walrus codegen reliably accepts only ONE sync wait per instruction
    (Matmult hard-fails at 2, Drain at 5). Tile's scheduler can attach more.
    Move excess waits onto preceding same-engine NOPs — semantically identical
    since engine queues execute in order."""
    from concourse import mybir

    for f in nc.m.functions:
        for blk in f.blocks:
            il = blk.instructions
            i = 0
            while i < len(il):
                inst = il[i]
                si = inst.sync_info
                waits = list(si.on_wait) if si is not None and si.on_wait else []
                if len(waits) > 1:
                    keep = waits[-1:]
                    excess = waits[:-1]
                    pos = i
                    for j, wcond in enumerate(excess):
                        nop = mybir.InstNoOp(name=f"{inst.name}-ws{j}", ins=[], outs=[])
                        nop.engine = inst.engine
                        nop.sync_info = mybir.SyncInfo(on_wait=[wcond], on_update=[])
                        il.insert(pos, nop)
                        pos += 1
                        i += 1
                    inst.sync_info = mybir.SyncInfo(
                        on_wait=keep,
                        on_update=list(si.on_update) if si.on_update else [],
                    )
                i += 1


def _build(cfg, waitsplit=True, debug_out=None):
    import concourse.bass as bass
    import concourse.tile as tile
    from concourse import mybir

    F32 = mybir.dt.float32
    F32R = mybir.dt.float32r
    AF = mybir.ActivationFunctionType
    MULT = mybir.AluOpType.mult

    L, D, HC, HD = cfg["L"], cfg["D"], cfg["HC"], cfg["HD"]
    DCH = D // 128            # contraction chunks for projections
    QKC = 2 * HC * HD // 128  # qkT M-chunks (q then k)
    KQ2 = QKC // 2            # chunks holding q (first half)
    LC = L // 512             # proj L chunks
    KC = L // 128             # attention k chunks
    QH = L // 1024            # attention q hemis (1024 wide)
    DV = HC * HD              # v width per core
    PAIRS = HC // 2
    DOUT = D
    DC = DOUT // 512

    from concourse.tile_rust import add_dep_helper

    nc = bass.Bass()
    xT_d = nc.dram_tensor("xT", [D, L], F32, kind="ExternalInput")
    wqk_d = nc.dram_tensor("wqk", [128, DCH, 2 * DV], F32, kind="ExternalInput")
    wv_d = nc.dram_tensor("wv", [128, DCH, DV], F32, kind="ExternalInput")
    bqk_d = nc.dram_tensor("bqk", [128, QKC], F32, kind="ExternalInput")
    bv_d = nc.dram_tensor("bv", [1, DV], F32, kind="ExternalInput")
    mb_d = nc.dram_tensor("mb", [128, KC], F32, kind="ExternalInput")
    wo_d = nc.dram_tensor("wo", [64, HC, DOUT], F32, kind="ExternalInput")
    bo_d = nc.dram_tensor("bo", [1, DOUT], F32, kind="ExternalInput")
    y_d = nc.dram_tensor("y", [L, DOUT], F32, kind="ExternalOutput")
    otn_d = None
    if debug_out == "otn":
        otn_d = nc.dram_tensor("otn_o", [64, HC, L], F32, kind="ExternalOutput")

    with tile.TileContext(nc) as tc, \
         nc.allow_low_precision(reason="float32r matmuls intended"):
        with tc.tile_pool(name="const", bufs=1) as const, \
             tc.tile_pool(name="big", bufs=1) as big, \
             tc.tile_pool(name="workP", bufs=4) as workP, \
             tc.tile_pool(name="workS", bufs=2) as workS:

            # ---- constants ----
            bqk_t = const.tile([128, QKC], F32)
            nc.sync.dma_start(out=bqk_t, in_=bqk_d[:, :])
            mb_t = const.tile([128, KC], F32)
            nc.sync.dma_start(out=mb_t, in_=mb_d[:, :])
            bv_r = const.tile([1, DV], F32R)
            nc.gpsimd.dma_start(out=bv_r, in_=bv_d[:, :])
            bo_r = const.tile([1, DOUT], F32R)
            nc.gpsimd.dma_start(out=bo_r, in_=bo_d[:, :])
            ones_f = const.tile([128, 1], F32)
            nc.vector.memset(ones_f, 1.0)
            ones_r = const.tile([1, 128], F32R)
            nc.vector.tensor_copy(out=ones_r, in_=ones_f[0:1, 0:1].broadcast_to([1, 128]))

            # ---- persistent big tensors ----
            qkT_r = big.tile([128, QKC, L], F32R)          # q then k, transposed
            vp_r = big.tile([128, KC, HC, HD + 1], F32R)   # v' with ones column
            otn_r = big.tile([64, HC, L], F32R)            # normalized O^T per head

            # ones column of v' (read by every av matmul)
            nc.vector.tensor_copy(
                out=vp_r[:, :, :, HD:HD + 1],
                in_=ones_f.unsqueeze(1).unsqueeze(1).broadcast_to([128, KC, HC, 1]),
            )

            # ---- projections ----
            with tc.tile_pool(name="w", bufs=1) as wpool, \
                 tc.tile_pool(name="xt", bufs=2) as xtpool, \
                 tc.tile_pool(name="psA", bufs=2, space="PSUM") as psA:
                wqk_r = wpool.tile([128, DCH, 2 * DV], F32R)
                nc.gpsimd.dma_start(out=wqk_r, in_=wqk_d[:, :, :])
                wv_r = wpool.tile([128, DCH, DV], F32R)
                nc.gpsimd.dma_start(out=wv_r, in_=wv_d[:, :, :])

                for lc in range(LC):
                    xt_r = xtpool.tile([128, DCH, 512], F32R)
                    nc.gpsimd.dma_start(
                        out=xt_r,
                        in_=xT_d.rearrange("(c p) l -> p c l", p=128)[:, :, lc * 512:(lc + 1) * 512],
                    )
                    # qkT: out [Mc cols 128, 512 L] accumulated over D
                    for mc in range(QKC):
                        qk_ps = psA.tile([128, 512], F32, tag="qk")
                        for k in range(DCH):
                            nc.tensor.matmul(
                                qk_ps[:, :],
                                wqk_r[:, k, mc * 128:(mc + 1) * 128],
                                xt_r[:, k, :],
                                start=(k == 0), stop=(k == DCH - 1),
                            )
                        nc.vector.tensor_scalar_add(
                            out=qkT_r[:, mc, lc * 512:(lc + 1) * 512],
                            in0=qk_ps, scalar1=bqk_t[:, mc:mc + 1],
                        )
                    # v: out [L rows 128, DV] accumulated over D, + bias row
                    for sub in range(4):
                        v_ps = psA.tile([128, DV], F32, tag="v")
                        for k in range(DCH):
                            nc.tensor.matmul(
                                v_ps[:, :],
                                xt_r[:, k, sub * 128:(sub + 1) * 128],
                                wv_r[:, k, :],
                                start=(k == 0), stop=False,
                            )
                        nc.tensor.matmul(
                            v_ps[:, :], ones_r[0:1, :], bv_r[0:1, :],
                            start=False, stop=True,
                        )
                        kcg = lc * 4 + sub
                        nc.vector.tensor_copy(
                            out=vp_r[:, kcg, :, 0:HD],
                            in_=v_ps.rearrange("p (h d) -> p h d", h=HC),
                        )

            # ---- attention ----
            with tc.tile_pool(name="psB", bufs=2, space="PSUM") as psB, \
                 tc.tile_pool(name="psC", bufs=2, space="PSUM") as psC:
                for h in range(HC):
                    base = (h % 2) * 64
                    kchunk = KQ2 + h // 2
                    qchunk = h // 2
                    for qh in range(QH):
                        q0 = qh * 1024
                        ot_ps = psC.tile([HD + 1, 1024], F32, tag="ot")
                        for kc in range(KC):
                            st_ps = psB.tile([128, 1024], F32, tag="st")
                            for s in range(2):
                                nc.tensor.matmul(
                                    st_ps[:, s * 512:(s + 1) * 512],
                                    qkT_r[base:base + 64, kchunk, kc * 128:(kc + 1) * 128],
                                    qkT_r[base:base + 64, qchunk, q0 + s * 512:q0 + (s + 1) * 512],
                                    start=True, stop=True,
                                )
                            pT = workP.tile([128, 1024], F32R, tag="pT")
                            nc.scalar.activation(
                                out=pT, in_=st_ps, func=AF.Exp,
                                bias=mb_t[:, kc:kc + 1], scale=1.0,
                            )
                            for s in range(2):
                                nc.tensor.matmul(
                                    ot_ps[:, s * 512:(s + 1) * 512],
                                    vp_r[:, kc, h, :],
                                    pT[:, s * 512:(s + 1) * 512],
                                    start=(kc == 0), stop=(kc == KC - 1),
                                )
                        # normalize: otn = ot[0:64] * (1/sums) broadcast
                        recip_r = workS.tile([1, 1024], F32R, tag="recip")
                        nc.vector.reciprocal(out=recip_r, in_=ot_ps[HD:HD + 1, :])
                        bc_ps = psB.tile([64, 1024], F32, tag="st")
                        for s in range(2):
                            nc.tensor.matmul(
                                bc_ps[:, s * 512:(s + 1) * 512],
                                ones_r[0:1, 0:64],
                                recip_r[0:1, s * 512:(s + 1) * 512],
                                start=True, stop=True,
                            )
                        bc_sb = workS.tile([64, 1024], F32, tag="bc")
                        nc.vector.tensor_copy(out=bc_sb, in_=bc_ps)
                        nc.vector.tensor_tensor(
                            out=otn_r[:, h, q0:q0 + 1024],
                            in0=ot_ps[0:HD, :], in1=bc_sb, op=MULT,
                        )

            if debug_out == "otn":
                otn_f = big.tile([64, HC, L], F32)
                nc.vector.tensor_copy(out=otn_f, in_=otn_r)
                nc.sync.dma_start(out=otn_d[:, :, :], in_=otn_f)
                nc.vector.memset(ones_f, 1.0)  # keep y unwritten path harmless
            # ---- output projection ----
            if debug_out == "stop_after_attn":
                skip_outproj = True
            else:
                skip_outproj = False
            with tc.tile_pool(name="wo", bufs=1) as wopool, \
                 tc.tile_pool(name="psD", bufs=4, space="PSUM") as psD:
                wo_r = wopool.tile([64, HC, DOUT], F32R)
                nc.gpsimd.dma_start(out=wo_r, in_=wo_d[:, :, :])
                for qt in range(0 if skip_outproj else L // 128):
                    y_sb = workS.tile([128, DOUT], F32, tag="y")
                    for dc in range(DC):
                        y_ps = psD.tile([128, 512], F32, tag="y")
                        for h in range(HC):
                            nc.tensor.matmul(
                                y_ps[:, :],
                                otn_r[:, h, qt * 128:(qt + 1) * 128],
                                wo_r[:, h, dc * 512:(dc + 1) * 512],
                                start=(h == 0), stop=False,
                            )
                        nc.tensor.matmul(
                            y_ps[:, :], ones_r[0:1, :], bo_r[0:1, dc * 512:(dc + 1) * 512],
                            start=False, stop=True,
                        )
                        nc.vector.tensor_copy(
                            out=y_sb[:, dc * 512:(dc + 1) * 512], in_=y_ps,
                        )
                    nc.sync.dma_start(out=y_d[qt * 128:(qt + 1) * 128, :], in_=y_sb)

    # split multi-waits (walrus allows 1 sync wait per instruction reliably)
    if waitsplit:
        _split_excess_waits(nc)
    return nc


def _prep_inputs(x, mask, W_qkv, b_qkv, W_out, b_out, cfg):
    """Build the 8 per-core input maps (host-side shuffles, float32)."""
    L, D, HC, HD = cfg["L"], cfg["D"], cfg["HC"], cfg["HD"]
    DV = HC * HD
    N = x.shape[0]
    scale = 1.0 / np.sqrt(HD)
    Wt = np.ascontiguousarray(W_qkv.T).astype(np.float32)    # [D, 3D]
    WoT = np.ascontiguousarray(W_out.T).astype(np.float32)   # [D, D]
    DCH = D // 128
    QKC = 2 * DV // 128
    KC = L // 128
    PAIRS = HC // 2

    per_hg = []
    for hg in range(2):
        qs, ks, vs = hg * DV, D + hg * DV, 2 * D + hg * DV
        wqk = np.concatenate(
            [Wt[:, qs:qs + DV] * scale, Wt[:, ks:ks + DV]], axis=1
        )  # [D, 2DV]
        wqk = wqk.reshape(DCH, 128, 2 * DV)  # [c, p, cols]
        wqk = np.ascontiguousarray(wqk.transpose(1, 0, 2))  # [128, c, cols]
        wv = Wt[:, vs:vs + DV].reshape(DCH, 128, DV)
        wv = np.ascontiguousarray(wv.transpose(1, 0, 2))
        bqk = np.concatenate(
            [b_qkv[qs:qs + DV] * scale, b_qkv[ks:ks + DV]]
        ).reshape(QKC, 128)
        bqk = np.ascontiguousarray(bqk.T)  # [128, QKC]
        bv = np.ascontiguousarray(b_qkv[vs:vs + DV][None, :])
        # wo: [HD, HC, D] — per-head rows at partition base 0
        wo_heads = WoT[hg * DV:(hg + 1) * DV, :].reshape(HC, HD, D)
        wo = np.ascontiguousarray(wo_heads.transpose(1, 0, 2))
        per_hg.append(dict(wqk=wqk, wv=wv, bqk=bqk, bv=bv, wo=wo))

    # b_out only on hg=0 cores; partials are summed on host (avoid 2x bias)
    bo_full = np.ascontiguousarray(b_out[None, :]).astype(np.float32)
    bo_zero = np.zeros_like(bo_full)
    xTs, mbs = [], []
    for n in range(N):
        xTs.append(np.ascontiguousarray(x[n].T).astype(np.float32))
        mb = np.where(mask[n], np.float32(-1e9), np.float32(0.0))
        mbs.append(np.ascontiguousarray(mb.reshape(KC, 128).T))

    in_maps = []
    for c in range(2 * N):
        n, hg = c // 2, c % 2
        d = dict(per_hg[hg])
        d.update(xT=xTs[n], mb=mbs[n], bo=(bo_full if hg == 0 else bo_zero))
        in_maps.append(d)
    return in_maps


def kernel(x, mask, W_qkv, b_qkv, W_out, b_out):
    from concourse.bass_utils import run_bass_kernel_spmd

    x = np.asarray(x, dtype=np.float32)
    mask = np.asarray(mask)
    N, L, D = x.shape
    H = 16
    HD = D // H
    cfg = {"L": L, "D": D, "HC": H // 2, "HD": HD}

    key = (L, D, H)
    if key not in _KERNEL_CACHE:
        _KERNEL_CACHE[key] = _build(cfg)
    nc = _KERNEL_CACHE[key]

    in_maps = _prep_inputs(
        x, mask,
        np.asarray(W_qkv, np.float32), np.asarray(b_qkv, np.float32),
        np.asarray(W_out, np.float32), np.asarray(b_out, np.float32), cfg,
    )
    res = run_bass_kernel_spmd(nc, in_maps, list(range(2 * N)))
    out = np.empty((N, L, D), np.float32)
    for n in range(N):
        out[n] = res.results[2 * n]["y"] + res.results[2 * n + 1]["y"]
    return out
